# revision 9
# baseline (speedup 1.0000x reference)
"""nn_LocalGraph kernel: data-parallel across 8 NeuronCores.

Shards the batch axis (B=8) across the 8 cores, one batch element per
core; the small MLP weights are replicated. Pooling is within the node
axis, so no cross-core communication is needed. Accepts FULL inputs and
returns the FULL output.

Host<->device transfer over the axon tunnel dominates wall-clock, so:
  - input_states is shipped as bf16, pre-transposed to feature-major
    [B, D, M*N] (4MB instead of 8MB),
  - weights are packed into one flat bf16 buffer,
  - only [B, 2H, M] of the output is fetched as bf16: the final stage
    satisfies max_n(exclude_self_max(e)) == max_n(e), so the full
    output is tile(max_n e, 2), reconstructed on the host.

Device compute is a Bass/Tile kernel (layout: features on partitions,
rows on the free axis; 64 tiles of 512 rows per core):
  - h = W^T x on PE with the weights stationary; the LayerNorm mean is
    a linear functional of x (mu = mean_f(W^T x) = wsum^T x for b == 0)
    computed by a second single-column matmul on x, then subtracted via
    a K=1 accumulating matmul of (-mu) broadcast across partitions.
  - variance is taken after the mean subtract: sd = sqrt(sumsq/F + eps)
    with sumsq from an ACT Square pass + ones-column matmul.
  - 1/sd is only applied where pooling needs true values (layers 1, 3):
    a per-column positive scale passes through LN+ReLU unchanged
    (relu(c*x) = c*relu(x), LN(c*x) = LN(x) up to an O(eps) term).
  - exclude-self-max over the node axis runs on the free axis with
    exact duplicate-max handling; max(excl, x - 10000) == excl exactly
    because post-ReLU activations are >= 0.
The Bass kernel assumes b == 0, g == 1, be == 0 (what setup_inputs()
produces — verified per call); other inputs and any Bass failure fall
back to an XLA (jnp) device pipeline, then to CPU.
"""
import numpy as np
import jax
import jax.numpy as jnp
import ml_dtypes

EPS = 1e-5

# Hardcoded problem shape (nn_LocalGraph_21646635172634):
# input_states [B=8, M=128, N=256, D_IN=8]; hidden H=64; output [8, 128, 256].
N_CORES = 8
B, M, N, D_IN, H = 8, 128, 256, 8, 64
MN = M * N                       # 32768 rows per core
R = 512                          # rows per tile (2 instances)
NT = MN // R                     # 64 tiles per core
KS = [D_IN, H, 2 * H, 2 * H]     # matmul contraction dims per layer
FS = [H, H, 2 * H, 2 * H]        # matmul output dims per layer
BF16 = ml_dtypes.bfloat16

_ARG_NAMES = ["input_states"] + [
    f"{p}{i}" for i in range(4) for p in ("W", "b", "g", "be")
]

# Packed-weights layout (bf16): per layer W (K*F row-major), wsum (K).
_WOFF = {}
_p = 0
for _l in range(4):
    for _nm, _sz in [("W", KS[_l] * FS[_l]), ("ws", KS[_l])]:
        _WOFF[(_nm, _l)] = (_p, _p + _sz)
        _p += _sz
WLEN = _p


def _pack_weights(inputs):
    w = np.empty((WLEN,), dtype=BF16)
    for l in range(4):
        Wl = np.asarray(inputs[f"W{l}"], dtype=np.float32)
        a, b = _WOFF[("W", l)]
        w[a:b] = Wl.reshape(-1).astype(BF16)
        a, b = _WOFF[("ws", l)]
        w[a:b] = Wl.sum(axis=1).astype(BF16)
    return w


def _is_fast(inputs):
    for l in range(4):
        if not (np.all(np.asarray(inputs[f"b{l}"]) == 0.0)
                and np.all(np.asarray(inputs[f"g{l}"]) == 1.0)
                and np.all(np.asarray(inputs[f"be{l}"]) == 0.0)):
            return False
    return True


# ---------------------------------------------------------------------------
# Bass kernel (per core, b==0 / g==1 / be==0)
def _build_nc():
    import concourse.bass as bass
    import concourse.tile as tile
    import concourse.mybir as mybir
    from contextlib import ExitStack

    f32 = mybir.dt.float32
    bf16 = mybir.dt.bfloat16
    Alu = mybir.AluOpType
    Act = mybir.ActivationFunctionType
    AX = mybir.AxisListType

    nc = bass.Bass(trn_type="TRN2")
    xd = nc.dram_tensor("x", [D_IN, MN], bf16, kind="ExternalInput")
    wd = nc.dram_tensor("wp", [WLEN], bf16, kind="ExternalInput")
    od = nc.dram_tensor("out", [2 * H, M], bf16, kind="ExternalOutput")

    def wslice(nm, l, cols):
        a, b = _WOFF[(nm, l)]
        return wd[a:b].rearrange("(k f) -> k f", f=cols)

    with tile.TileContext(nc) as tc, ExitStack() as ctx:
        consts = ctx.enter_context(tc.tile_pool(name="consts", bufs=1))
        p0p = ctx.enter_context(tc.tile_pool(name="p0p", bufs=2, space="PSUM"))
        pmup = ctx.enter_context(tc.tile_pool(name="pmup", bufs=2, space="PSUM"))
        pqp = ctx.enter_context(tc.tile_pool(name="pqp", bufs=2, space="PSUM"))
        pbp = ctx.enter_context(tc.tile_pool(name="pbp", bufs=2, space="PSUM"))
        work = ctx.enter_context(tc.tile_pool(name="work", bufs=3))
        stats = ctx.enter_context(tc.tile_pool(name="stats", bufs=4))
        xp = ctx.enter_context(tc.tile_pool(name="xp", bufs=3))

        # --- constants & weights (loaded once) ---
        xall = consts.tile([D_IN, MN], bf16)
        nc.sync.dma_start(out=xall, in_=xd[:, :])
        ones_row = consts.tile([1, 128], bf16)
        nc.vector.memset(ones_row, 1.0)
        ones_col = consts.tile([128, 1], bf16)
        nc.vector.memset(ones_col, 1.0)
        eps_sb = consts.tile([1, 1], f32)
        nc.vector.memset(eps_sb, EPS)

        Wt, WSt = [], []
        for l in range(4):
            Wl = consts.tile([KS[l], FS[l]], bf16, tag=f"W{l}")
            nc.sync.dma_start(out=Wl, in_=wslice("W", l, FS[l]))
            ws = consts.tile([KS[l], 1], bf16, tag=f"ws{l}")
            nc.sync.dma_start(out=ws, in_=wslice("ws", l, 1))
            Wt.append(Wl)
            WSt.append(ws)

        out_sb = consts.tile([2 * H, M], bf16)

        def emit_mlp(l, xin, use_inv):
            """Linear + LN(+ReLU) on a [K, R] input tile.

            Returns (w, pb): w = relu(h - mu) [F, R] in SBUF and, when
            use_inv, pb = (1/sd) broadcast [F, R] in PSUM (else None) —
            the caller multiplies them where true values are needed.
            """
            F = FS[l]
            pmu = pmup.tile([1, R], f32, tag="pmu")
            nc.tensor.matmul(pmu, WSt[l], xin, start=True, stop=True)
            nmu = stats.tile([1, R], bf16, tag="nmu")
            nc.scalar.activation(nmu, pmu, Act.Copy, scale=-1.0 / F)
            h = p0p.tile([F, R], f32, tag="p0")
            nc.tensor.matmul(h, Wt[l], xin, start=True, stop=False)
            nc.tensor.matmul(h, ones_row[:, :F], nmu, start=False, stop=True)

            pb = None
            if use_inv:
                hsq = work.tile([F, R], bf16, tag="hsq")
                nc.scalar.square(hsq, h)
                pq = pqp.tile([1, R], f32, tag="pq")
                nc.tensor.matmul(pq, ones_col[:F, :], hsq, start=True, stop=True)
                sd = stats.tile([1, R], f32, tag="sd")
                nc.scalar.activation(sd, pq, Act.Sqrt, scale=1.0 / F,
                                     bias=eps_sb)
                inv = stats.tile([1, R], f32, tag="inv")
                nc.vector.reciprocal(inv, sd)
                inv16 = stats.tile([1, R], bf16, tag="inv16")
                nc.scalar.activation(inv16, inv, Act.Copy)
                pb = pbp.tile([F, R], f32, tag="pb")
                nc.tensor.matmul(pb, ones_row[:, :F], inv16, start=True,
                                 stop=True)

            w = work.tile([F, R], bf16, tag=f"w{l}")
            nc.scalar.activation(w, h, Act.Relu)
            return w, pb

        for t in range(NT):
            xin0 = xall[:, t * R:(t + 1) * R]
            # ---- layers 0, 1 (scale flows through 0; 1 feeds pooling) ----
            x1, _ = emit_mlp(0, xin0, use_inv=False)
            w1, pb1 = emit_mlp(1, x1, use_inv=True)
            # e1 = w1 * (1/sd), stacked: 2 instances x 64 feats -> [128, 256]
            e1st = work.tile([128, N], bf16, tag="e1st")
            nc.vector.tensor_mul(e1st[0:H, :], w1[:, 0:N], pb1[:, 0:N])
            nc.vector.tensor_mul(e1st[H:128, :], w1[:, N:R], pb1[:, N:R])
            # ---- exclude-self max over the node axis (free axis) ----
            m1 = stats.tile([128, 1], f32, tag="m1")
            nc.vector.reduce_max(m1, e1st, axis=AX.X)
            # eqm1 = (e1st == m1) - 1  in {0, -1}
            eqm1 = work.tile([128, N], bf16, tag="eqm1")
            nc.vector.tensor_scalar(eqm1, e1st, scalar1=m1, scalar2=1.0,
                                    op0=Alu.is_equal, op1=Alu.subtract)
            cnt = stats.tile([128, 1], f32, tag="cnt")
            nc.vector.reduce_sum(cnt, eqm1, axis=AX.X)
            # m2 = max over non-argmax nodes (exact: values are >= 0)
            scratch = work.tile([128, N], bf16, tag="scratch")
            m2 = stats.tile([128, 1], f32, tag="m2")
            nc.vector.tensor_tensor_reduce(
                out=scratch, in0=e1st, in1=eqm1, scale=-1.0, scalar=0.0,
                op0=Alu.mult, op1=Alu.max, accum_out=m2)
            uniq = stats.tile([128, 1], bf16, tag="uniq")
            nc.vector.tensor_scalar(uniq, cnt, scalar1=-(N - 1.0),
                                    scalar2=None, op0=Alu.is_equal)
            d = stats.tile([128, 1], f32, tag="d")
            nc.vector.tensor_tensor(d, m2, m1, op=Alu.subtract)
            ud = stats.tile([128, 1], f32, tag="ud")
            nc.vector.tensor_mul(ud, uniq, d)
            a2 = stats.tile([128, 1], f32, tag="a2")
            nc.vector.tensor_tensor(a2, m1, ud, op=Alu.add)
            # excl = eqm1*ud + (m1+ud): argmax -> m2 if unique else m1;
            # everyone else -> m1
            excl = work.tile([128, N], bf16, tag="excl")
            nc.vector.tensor_scalar(excl, eqm1, scalar1=ud, scalar2=a2,
                                    op0=Alu.mult, op1=Alu.add)
            # ---- assemble layer-2 input [128, R] = [e1; excl] ----
            x2 = xp.tile([2 * H, R], bf16, tag="x2")
            nc.gpsimd.tensor_copy(out=x2[0:H, 0:N], in_=e1st[0:H, :])
            nc.gpsimd.tensor_copy(out=x2[0:H, N:R], in_=e1st[H:128, :])
            nc.gpsimd.tensor_copy(out=x2[H:128, 0:N], in_=excl[0:H, :])
            nc.gpsimd.tensor_copy(out=x2[H:128, N:R], in_=excl[H:128, :])
            # ---- layers 2, 3 + final max over nodes ----
            x3, _ = emit_mlp(2, x2, use_inv=False)
            w3, pb3 = emit_mlp(3, x3, use_inv=True)
            e3 = work.tile([2 * H, R], bf16, tag="e3")
            nc.vector.tensor_mul(e3, w3, pb3)
            nc.vector.reduce_max(
                out_sb[:, 2 * t:2 * t + 2],
                e3.rearrange("p (i n) -> p i n", n=N), axis=AX.X)

        nc.sync.dma_start(out=od[:, :], in_=out_sb)
    return nc


# ---------------------------------------------------------------------------
# Execution wrappers
_STATE = {}


def _get_bass_call():
    if "bass" not in _STATE:
        from concourse import bass2jax, mybir
        from jax.experimental.shard_map import shard_map
        from jax.sharding import Mesh, PartitionSpec
        bass2jax.install_neuronx_cc_hook()
        nc = _build_nc()

        in_names, out_names, out_avals, zero_shapes = [], [], [], []
        for alloc in nc.m.functions[0].allocations:
            if not isinstance(alloc, mybir.MemoryLocationSet):
                continue
            name = alloc.memorylocations[0].name
            if alloc.kind == "ExternalInput":
                in_names.append(name)
            elif alloc.kind == "ExternalOutput":
                out_names.append(name)
                shape = tuple(alloc.tensor_shape)
                dtype = mybir.dt.np(alloc.dtype)
                out_avals.append(jax.core.ShapedArray(shape, dtype))
                zero_shapes.append((shape, dtype))
        n_params = len(in_names)
        all_names = tuple(in_names + out_names)

        def _body(*args):
            return tuple(bass2jax._bass_exec_p.bind(
                *args,
                out_avals=tuple(out_avals),
                in_names=all_names,
                out_names=tuple(out_names),
                lowering_input_output_aliases=(),
                sim_require_finite=True,
                sim_require_nnan=True,
                nc=nc,
            ))

        devs = jax.devices()[:N_CORES]
        mesh = Mesh(np.asarray(devs), ("core",))
        nin = n_params + len(out_names)
        sharded = jax.jit(
            shard_map(_body, mesh=mesh,
                      in_specs=(PartitionSpec("core"),) * nin,
                      out_specs=(PartitionSpec("core"),) * len(out_names),
                      check_rep=False),
            donate_argnums=tuple(range(n_params, nin)),
            keep_unused=True)
        _STATE["bass"] = (sharded, in_names, zero_shapes)
    return _STATE["bass"]


def _run_bass(inputs):
    sharded, in_names, zero_shapes = _get_bass_call()
    x = np.asarray(inputs["input_states"], dtype=np.float32)
    x16 = x.transpose(0, 3, 1, 2).astype(BF16)          # [B, D, M, N]
    wp = _pack_weights(inputs)
    glob = {"x": x16.reshape(B * D_IN, MN),
            "wp": np.tile(wp, B)}
    args = [glob[nm] for nm in in_names]
    zeros = [np.zeros((B * s[0],) + s[1:], dt) for s, dt in zero_shapes]
    out = sharded(*args, *zeros)[0]
    o = np.asarray(out).reshape(B, 2 * H, M).astype(np.float32)
    o = o.transpose(0, 2, 1)                            # [B, M, 2H]
    return np.concatenate([o, o], axis=-1)              # [B, M, 4H]


# ---------------------------------------------------------------------------
# XLA fallback (same math in jnp, arbitrary b/g/be)
def _mlp(x, W, b, g, be):
    h = x @ W + b
    mu = jnp.mean(h, axis=-1, keepdims=True)
    var = jnp.var(h, axis=-1, keepdims=True)
    h = (h - mu) * jax.lax.rsqrt(var + EPS) * g + be
    return jax.nn.relu(h)


def _exclude_self_max(x):
    m1 = jnp.max(x, axis=-2, keepdims=True)
    eq = x == m1
    unique = jnp.sum(eq, axis=-2, keepdims=True) == 1
    m2 = jnp.max(jnp.where(eq, -3.0e38, x), axis=-2, keepdims=True)
    excl = jnp.where(eq & unique, m2, m1)
    return jnp.maximum(excl, x - 10000.0)


def _forward(x16, W0, b0, g0, be0, W1, b1, g1, be1,
             W2, b2, g2, be2, W3, b3, g3, be3):
    x = x16.astype(jnp.float32)
    e = _mlp(_mlp(x, W0, b0, g0, be0), W1, b1, g1, be1)
    e = jnp.concatenate([e, _exclude_self_max(e)], axis=-1)
    e = _mlp(_mlp(e, W2, b2, g2, be2), W3, b3, g3, be3)
    return jnp.max(e, axis=-2).astype(jnp.bfloat16)     # [..., M, 2H]


def _half_to_full(out16):
    half = np.asarray(out16).astype(np.float32)
    return np.concatenate([half, half], axis=-1)


def _run_xla(inputs):
    from jax.sharding import Mesh, PartitionSpec, NamedSharding
    if "xla" not in _STATE:
        devs = jax.devices()[:N_CORES]
        mesh = Mesh(np.asarray(devs), ("b",))
        shard = NamedSharding(mesh, PartitionSpec("b"))
        repl = NamedSharding(mesh, PartitionSpec())
        _STATE["xla"] = jax.jit(
            _forward, in_shardings=(shard,) + (repl,) * 16,
            out_shardings=shard)
    x16 = np.asarray(inputs["input_states"]).astype(BF16)
    args = [np.asarray(inputs[nm], np.float32) for nm in _ARG_NAMES[1:]]
    return _half_to_full(_STATE["xla"](x16, *args))


def _run_cpu(inputs):
    cpu = jax.devices("cpu")[0]
    with jax.default_device(cpu):
        x16 = np.asarray(inputs["input_states"]).astype(BF16)
        args = [np.asarray(inputs[nm], np.float32) for nm in _ARG_NAMES[1:]]
        return _half_to_full(jax.jit(_forward)(x16, *args))


def kernel(**inputs):
    if _is_fast(inputs):
        try:
            return _run_bass(inputs).astype(np.float32)
        except Exception:
            pass
    try:
        return _run_xla(inputs).astype(np.float32)
    except Exception:
        return _run_cpu(inputs).astype(np.float32)


# revision 12
# speedup vs baseline: 1.1862x; 1.1862x over previous
"""nn_LocalGraph kernel: data-parallel across 8 NeuronCores.

Shards the batch axis (B=8) across the 8 cores, one batch element per
core; the small MLP weights are replicated. Pooling is within the node
axis, so no cross-core communication is needed. Accepts FULL inputs and
returns the FULL output.

Host<->device transfer over the axon tunnel dominates wall-clock, so:
  - input_states is shipped as bf16, pre-transposed to feature-major
    [B, D, M*N] (4MB instead of 8MB),
  - weights are packed into one flat bf16 buffer,
  - only [B, 2H, M] of the output is fetched as bf16: the final stage
    satisfies max_n(exclude_self_max(e)) == max_n(e), so the full
    output is tile(max_n e, 2), reconstructed on the host.

Device compute is a Bass/Tile kernel (layout: features on partitions,
rows on the free axis; 64 tiles of 512 rows per core):
  - h = W^T x on PE with the weights stationary; the LayerNorm mean is
    a linear functional of x (mu = mean_f(W^T x) = wsum^T x for b == 0)
    computed by a second single-column matmul on x, then subtracted via
    a K=1 accumulating matmul of (-mu) broadcast across partitions.
  - variance is taken after the mean subtract: sd = sqrt(sumsq/F + eps)
    with sumsq from an ACT Square pass + ones-column matmul.
  - 1/sd is only applied where pooling needs true values (layers 1, 3):
    a per-column positive scale passes through LN+ReLU unchanged
    (relu(c*x) = c*relu(x), LN(c*x) = LN(x) up to an O(eps) term).
  - exclude-self-max over the node axis runs on the free axis with
    exact duplicate-max handling; max(excl, x - 10000) == excl exactly
    because post-ReLU activations are >= 0.
The Bass kernel assumes b == 0, g == 1, be == 0 (what setup_inputs()
produces — verified per call); other inputs and any Bass failure fall
back to an XLA (jnp) device pipeline, then to CPU.
"""
import numpy as np
import jax
import jax.numpy as jnp
import ml_dtypes

EPS = 1e-5

# Hardcoded problem shape (nn_LocalGraph_21646635172634):
# input_states [B=8, M=128, N=256, D_IN=8]; hidden H=64; output [8, 128, 256].
N_CORES = 8
B, M, N, D_IN, H = 8, 128, 256, 8, 64
MN = M * N                       # 32768 rows per core
R = 512                          # rows per tile (2 instances)
NT = MN // R                     # 64 tiles per core
KS = [D_IN, H, 2 * H, 2 * H]     # matmul contraction dims per layer
FS = [H, H, 2 * H, 2 * H]        # matmul output dims per layer
BF16 = ml_dtypes.bfloat16

_ARG_NAMES = ["input_states"] + [
    f"{p}{i}" for i in range(4) for p in ("W", "b", "g", "be")
]

# Packed-weights layout (bf16): per layer W (K*F row-major), wsum (K).
_WOFF = {}
_p = 0
for _l in range(4):
    for _nm, _sz in [("W", KS[_l] * FS[_l]), ("ws", KS[_l])]:
        _WOFF[(_nm, _l)] = (_p, _p + _sz)
        _p += _sz
WLEN = _p


def _pack_weights(inputs):
    w = np.empty((WLEN,), dtype=BF16)
    for l in range(4):
        Wl = np.asarray(inputs[f"W{l}"], dtype=np.float32)
        a, b = _WOFF[("W", l)]
        w[a:b] = Wl.reshape(-1).astype(BF16)
        a, b = _WOFF[("ws", l)]
        w[a:b] = Wl.sum(axis=1).astype(BF16)
    return w


def _is_fast(inputs):
    for l in range(4):
        if not (np.all(np.asarray(inputs[f"b{l}"]) == 0.0)
                and np.all(np.asarray(inputs[f"g{l}"]) == 1.0)
                and np.all(np.asarray(inputs[f"be{l}"]) == 0.0)):
            return False
    return True


# ---------------------------------------------------------------------------
# Bass kernel (per core, b==0 / g==1 / be==0)
def _build_nc():
    import concourse.bass as bass
    import concourse.tile as tile
    import concourse.mybir as mybir
    from contextlib import ExitStack

    f32 = mybir.dt.float32
    bf16 = mybir.dt.bfloat16
    Alu = mybir.AluOpType
    Act = mybir.ActivationFunctionType
    AX = mybir.AxisListType

    nc = bass.Bass(trn_type="TRN2", enable_partition_id=False)
    xd = nc.dram_tensor("x", [D_IN, MN], bf16, kind="ExternalInput")
    wd = nc.dram_tensor("wp", [WLEN], bf16, kind="ExternalInput")
    od = nc.dram_tensor("out", [2 * H, M], bf16, kind="ExternalOutput")

    def wslice(nm, l, cols):
        a, b = _WOFF[(nm, l)]
        return wd[a:b].rearrange("(k f) -> k f", f=cols)

    with tile.TileContext(nc) as tc, ExitStack() as ctx:
        consts = ctx.enter_context(tc.tile_pool(name="consts", bufs=1))
        p0p = ctx.enter_context(tc.tile_pool(name="p0p", bufs=2, space="PSUM"))
        pmup = ctx.enter_context(tc.tile_pool(name="pmup", bufs=2, space="PSUM"))
        pqp = ctx.enter_context(tc.tile_pool(name="pqp", bufs=2, space="PSUM"))
        pbp = ctx.enter_context(tc.tile_pool(name="pbp", bufs=2, space="PSUM"))
        work = ctx.enter_context(tc.tile_pool(name="work", bufs=3))
        stats = ctx.enter_context(tc.tile_pool(name="stats", bufs=4))
        xp = ctx.enter_context(tc.tile_pool(name="xp", bufs=3))

        # --- constants & weights (loaded once) ---
        xall = consts.tile([D_IN, MN], bf16)
        nc.sync.dma_start(out=xall, in_=xd[:, :])
        ones_row = consts.tile([1, 128], bf16)
        nc.vector.memset(ones_row, 1.0)
        ones_col = consts.tile([128, 1], bf16)
        nc.vector.memset(ones_col, 1.0)
        eps_sb = consts.tile([1, 1], f32)
        nc.vector.memset(eps_sb, EPS)

        Wt, WSt = [], []
        for l in range(4):
            Wl = consts.tile([KS[l], FS[l]], bf16, tag=f"W{l}")
            nc.sync.dma_start(out=Wl, in_=wslice("W", l, FS[l]))
            ws = consts.tile([KS[l], 1], bf16, tag=f"ws{l}")
            nc.sync.dma_start(out=ws, in_=wslice("ws", l, 1))
            Wt.append(Wl)
            WSt.append(ws)

        out_sb = consts.tile([2 * H, M], bf16)

        def emit_mlp(l, xin, use_inv):
            """Linear + LN(+ReLU) on a [K, R] input tile.

            Returns (w, pb): w = relu(h - mu) [F, R] in SBUF and, when
            use_inv, pb = (1/sd) broadcast [F, R] in PSUM (else None) —
            the caller multiplies them where true values are needed.
            """
            F = FS[l]
            pmu = pmup.tile([1, R], f32, tag="pmu")
            nc.tensor.matmul(pmu, WSt[l], xin, start=True, stop=True)
            nmu = stats.tile([1, R], bf16, tag="nmu")
            nc.scalar.activation(nmu, pmu, Act.Copy, scale=-1.0 / F)
            h = p0p.tile([F, R], f32, tag="p0")
            nc.tensor.matmul(h, Wt[l], xin, start=True, stop=False)
            nc.tensor.matmul(h, ones_row[:, :F], nmu, start=False, stop=True)

            pb = None
            if use_inv:
                hsq = work.tile([F, R], bf16, tag="hsq")
                nc.scalar.square(hsq, h)
                pq = pqp.tile([1, R], f32, tag="pq")
                nc.tensor.matmul(pq, ones_col[:F, :], hsq, start=True, stop=True)
                sd = stats.tile([1, R], f32, tag="sd")
                nc.scalar.activation(sd, pq, Act.Sqrt, scale=1.0 / F,
                                     bias=eps_sb)
                inv = stats.tile([1, R], f32, tag="inv")
                nc.vector.reciprocal(inv, sd)
                inv16 = stats.tile([1, R], bf16, tag="inv16")
                nc.scalar.activation(inv16, inv, Act.Copy)
                pb = pbp.tile([F, R], f32, tag="pb")
                nc.tensor.matmul(pb, ones_row[:, :F], inv16, start=True,
                                 stop=True)

            w = work.tile([F, R], bf16, tag=f"w{l}")
            nc.scalar.activation(w, h, Act.Relu)
            return w, pb

        for t in range(NT):
            xin0 = xall[:, t * R:(t + 1) * R]
            # ---- layers 0, 1 (scale flows through 0; 1 feeds pooling) ----
            x1, _ = emit_mlp(0, xin0, use_inv=False)
            w1, pb1 = emit_mlp(1, x1, use_inv=True)
            # e1 = w1 * (1/sd), stacked: 2 instances x 64 feats -> [128, 256]
            e1st = work.tile([128, N], bf16, tag="e1st")
            nc.vector.tensor_mul(e1st[0:H, :], w1[:, 0:N], pb1[:, 0:N])
            nc.vector.tensor_mul(e1st[H:128, :], w1[:, N:R], pb1[:, N:R])
            # ---- exclude-self max over the node axis (free axis) ----
            m1 = stats.tile([128, 1], f32, tag="m1")
            nc.vector.reduce_max(m1, e1st, axis=AX.X)
            # eqm1 = (e1st == m1) - 1  in {0, -1}
            eqm1 = work.tile([128, N], bf16, tag="eqm1")
            nc.vector.tensor_scalar(eqm1, e1st, scalar1=m1, scalar2=1.0,
                                    op0=Alu.is_equal, op1=Alu.subtract)
            cnt = stats.tile([128, 1], f32, tag="cnt")
            nc.vector.reduce_sum(cnt, eqm1, axis=AX.X)
            # m2 = max over non-argmax nodes (exact: values are >= 0)
            scratch = work.tile([128, N], bf16, tag="scratch")
            nc.vector.tensor_mul(scratch, e1st, eqm1)  # 0 at argmax, -x else
            m2n = stats.tile([128, 1], f32, tag="m2n")
            nc.vector.tensor_reduce(m2n, scratch, axis=AX.X, op=Alu.min)
            m2 = stats.tile([128, 1], f32, tag="m2")
            nc.vector.tensor_scalar_mul(m2, m2n, -1.0)
            uniq = stats.tile([128, 1], bf16, tag="uniq")
            nc.vector.tensor_scalar(uniq, cnt, scalar1=-(N - 1.0),
                                    scalar2=None, op0=Alu.is_equal)
            d = stats.tile([128, 1], f32, tag="d")
            nc.vector.tensor_tensor(d, m2, m1, op=Alu.subtract)
            ud = stats.tile([128, 1], f32, tag="ud")
            nc.vector.tensor_mul(ud, uniq, d)
            a2 = stats.tile([128, 1], f32, tag="a2")
            nc.vector.tensor_tensor(a2, m1, ud, op=Alu.add)
            # excl = eqm1*ud + (m1+ud): argmax -> m2 if unique else m1;
            # everyone else -> m1
            excl = work.tile([128, N], bf16, tag="excl")
            nc.vector.tensor_scalar(excl, eqm1, scalar1=ud, scalar2=a2,
                                    op0=Alu.mult, op1=Alu.add)
            # ---- assemble layer-2 input [128, R] = [e1; excl] ----
            x2 = xp.tile([2 * H, R], bf16, tag="x2")
            nc.gpsimd.tensor_copy(out=x2[0:H, 0:N], in_=e1st[0:H, :])
            nc.gpsimd.tensor_copy(out=x2[0:H, N:R], in_=e1st[H:128, :])
            nc.gpsimd.tensor_copy(out=x2[H:128, 0:N], in_=excl[0:H, :])
            nc.gpsimd.tensor_copy(out=x2[H:128, N:R], in_=excl[H:128, :])
            # ---- layers 2, 3 + final max over nodes ----
            x3, _ = emit_mlp(2, x2, use_inv=False)
            w3, pb3 = emit_mlp(3, x3, use_inv=True)
            e3 = work.tile([2 * H, R], bf16, tag="e3")
            nc.vector.tensor_mul(e3, w3, pb3)
            nc.vector.reduce_max(
                out_sb[:, 2 * t:2 * t + 2],
                e3.rearrange("p (i n) -> p i n", n=N), axis=AX.X)

        nc.sync.dma_start(out=od[:, :], in_=out_sb)
    return nc


# ---------------------------------------------------------------------------
# Execution wrappers
_STATE = {}


def _split_multi_waits(bir: bytes) -> bytes:
    """Rewrite BIR so no instruction carries more than one sync wait.

    The walrus build in this container encodes a single wait command per
    ISA instruction; extra waits are moved to standalone EventSemaphore
    instructions inserted just before (same engine, program order, so
    semantics are identical)."""
    import json as _json
    j = _json.loads(bir)
    ctr = [0]

    def fix_block(b):
        out = []
        for ins in b.get("instructions", []):
            si = ins.get("sync_info")
            waits = (si or {}).get("on_wait") or []
            if len(waits) > 1:
                for w in waits[:-1]:
                    ctr[0] += 1
                    out.append({
                        "debug": ins.get("debug", 0),
                        "engine": ins["engine"],
                        "ins": [], "outs": [],
                        "name": f"WSPLIT-{ctr[0]}",
                        "opcode": "EventSemaphore",
                        "sync_info": {"on_wait": [w], "on_update": []},
                    })
                si["on_wait"] = [waits[-1]]
            out.append(ins)
        b["instructions"] = out
        for sb in b.get("blocks", []):
            fix_block(sb)

    for fn in j["functions"]:
        for b in fn["blocks"]:
            fix_block(b)
    return _json.dumps(j).encode()


def _patch_compiler():
    import concourse.bass_utils as bass_utils
    import concourse.bass2jax as bass2jax
    if getattr(bass_utils, "_wait_split_patched", False):
        return
    orig = bass_utils.compile_bir_kernel

    def patched(bir_json, tmpdir, neff_name="file.neff"):
        if isinstance(bir_json, str):
            bir_json = bir_json.encode()
        return orig(_split_multi_waits(bir_json), tmpdir, neff_name=neff_name)

    bass_utils.compile_bir_kernel = patched
    bass2jax.compile_bir_kernel = patched
    bass_utils._wait_split_patched = True


def _get_bass_call():
    if "bass" not in _STATE:
        from concourse import bass2jax, mybir
        from jax.experimental.shard_map import shard_map
        from jax.sharding import Mesh, PartitionSpec
        bass2jax.install_neuronx_cc_hook()
        _patch_compiler()
        nc = _build_nc()

        in_names, out_names, out_avals, zero_shapes = [], [], [], []
        for alloc in nc.m.functions[0].allocations:
            if not isinstance(alloc, mybir.MemoryLocationSet):
                continue
            name = alloc.memorylocations[0].name
            if alloc.kind == "ExternalInput":
                in_names.append(name)
            elif alloc.kind == "ExternalOutput":
                out_names.append(name)
                shape = tuple(alloc.tensor_shape)
                dtype = mybir.dt.np(alloc.dtype)
                out_avals.append(jax.core.ShapedArray(shape, dtype))
                zero_shapes.append((shape, dtype))
        n_params = len(in_names)
        all_names = tuple(in_names + out_names)

        def _body(*args):
            return tuple(bass2jax._bass_exec_p.bind(
                *args,
                out_avals=tuple(out_avals),
                in_names=all_names,
                out_names=tuple(out_names),
                lowering_input_output_aliases=(),
                sim_require_finite=True,
                sim_require_nnan=True,
                nc=nc,
            ))

        devs = jax.devices()[:N_CORES]
        mesh = Mesh(np.asarray(devs), ("core",))
        nin = n_params + len(out_names)
        sharded = jax.jit(
            shard_map(_body, mesh=mesh,
                      in_specs=(PartitionSpec("core"),) * nin,
                      out_specs=(PartitionSpec("core"),) * len(out_names),
                      check_rep=False),
            donate_argnums=tuple(range(n_params, nin)),
            keep_unused=True)
        _STATE["bass"] = (sharded, in_names, zero_shapes)
    return _STATE["bass"]


def _run_bass(inputs):
    sharded, in_names, zero_shapes = _get_bass_call()
    x = np.asarray(inputs["input_states"], dtype=np.float32)
    x16 = x.transpose(0, 3, 1, 2).astype(BF16)          # [B, D, M, N]
    wp = _pack_weights(inputs)
    glob = {"x": x16.reshape(B * D_IN, MN),
            "wp": np.tile(wp, B)}
    args = [glob[nm] for nm in in_names]
    zeros = [np.zeros((B * s[0],) + s[1:], dt) for s, dt in zero_shapes]
    out = sharded(*args, *zeros)[0]
    o = np.asarray(out).reshape(B, 2 * H, M).astype(np.float32)
    o = o.transpose(0, 2, 1)                            # [B, M, 2H]
    return np.concatenate([o, o], axis=-1)              # [B, M, 4H]


# ---------------------------------------------------------------------------
# XLA fallback (same math in jnp, arbitrary b/g/be)
def _mlp(x, W, b, g, be):
    h = x @ W + b
    mu = jnp.mean(h, axis=-1, keepdims=True)
    var = jnp.var(h, axis=-1, keepdims=True)
    h = (h - mu) * jax.lax.rsqrt(var + EPS) * g + be
    return jax.nn.relu(h)


def _exclude_self_max(x):
    m1 = jnp.max(x, axis=-2, keepdims=True)
    eq = x == m1
    unique = jnp.sum(eq, axis=-2, keepdims=True) == 1
    m2 = jnp.max(jnp.where(eq, -3.0e38, x), axis=-2, keepdims=True)
    excl = jnp.where(eq & unique, m2, m1)
    return jnp.maximum(excl, x - 10000.0)


def _forward(x16, W0, b0, g0, be0, W1, b1, g1, be1,
             W2, b2, g2, be2, W3, b3, g3, be3):
    x = x16.astype(jnp.float32)
    e = _mlp(_mlp(x, W0, b0, g0, be0), W1, b1, g1, be1)
    e = jnp.concatenate([e, _exclude_self_max(e)], axis=-1)
    e = _mlp(_mlp(e, W2, b2, g2, be2), W3, b3, g3, be3)
    return jnp.max(e, axis=-2).astype(jnp.bfloat16)     # [..., M, 2H]


def _half_to_full(out16):
    half = np.asarray(out16).astype(np.float32)
    return np.concatenate([half, half], axis=-1)


def _run_xla(inputs):
    from jax.sharding import Mesh, PartitionSpec, NamedSharding
    if "xla" not in _STATE:
        devs = jax.devices()[:N_CORES]
        mesh = Mesh(np.asarray(devs), ("b",))
        shard = NamedSharding(mesh, PartitionSpec("b"))
        repl = NamedSharding(mesh, PartitionSpec())
        _STATE["xla"] = jax.jit(
            _forward, in_shardings=(shard,) + (repl,) * 16,
            out_shardings=shard)
    x16 = np.asarray(inputs["input_states"]).astype(BF16)
    args = [np.asarray(inputs[nm], np.float32) for nm in _ARG_NAMES[1:]]
    return _half_to_full(_STATE["xla"](x16, *args))


def _run_cpu(inputs):
    cpu = jax.devices("cpu")[0]
    with jax.default_device(cpu):
        x16 = np.asarray(inputs["input_states"]).astype(BF16)
        args = [np.asarray(inputs[nm], np.float32) for nm in _ARG_NAMES[1:]]
        return _half_to_full(jax.jit(_forward)(x16, *args))


def kernel(**inputs):
    if _is_fast(inputs):
        try:
            return _run_bass(inputs).astype(np.float32)
        except Exception:
            pass
    try:
        return _run_xla(inputs).astype(np.float32)
    except Exception:
        return _run_cpu(inputs).astype(np.float32)
